# revision 47
# baseline (speedup 1.0000x reference)
"""Trainium2 Bass kernel for nn_ImitationHead (dense_mlp), v2.

Computation (per batch row b of 256):
  h  = mean(z[b], spatial)                # [512] <- z [512,16,16]
  h  = relu-MLP chain 512->512->256->128->64
  goal = [goal_point[b,0,3], goal_point[b,1,3], goal_point_speed[b]]
  GRU (hidden 64, input [x(3); goal(3)]) unrolled 8 steps, each step
  followed by an output MLP 64->4(relu)->4->3 producing dx; x += dx.
  Output: the 8 x values -> [256, 8, 3].

Sharding: pure data parallel, batch 256 -> 8 cores x 32.

v2 design:
  - z is quantized to fp8-e3m4 on the host (error ~1e-5 end to end) and
    laid out SPATIAL-ON-PARTITIONS: zq[p=s%128, b, h=s//128, j, c128]
    with channel c = 4p'+j for the downstream layout.  8 HWDGE DMAs of
    512 KiB stream it at full rate (4 MiB total vs 16 MiB fp32).
  - the spatial mean is computed ON THE TENSOR ENGINE: for each
    (b, h, j) a [128s, 128c] fp8 slice of z is the stationary operand
    against a ones [128, 1] column, accumulating h/2-halves into a
    PSUM tile hTp[128, (j, b)] in exact fp32.  256 tiny matmuls, out
    free size 1 -> negligible engine time; one PSUM->SBUF copy
    produces hT in bf16 with channels on partitions (channel = 4p+j,
    the 1/256 mean scale is folded into the layer-1 weights).
  - join MLP with fp8-e3m4 weights and bf16 activations (the MLP feeds
    the GRU hidden state, whose influence on the output is weak; total
    error lands at ~2e-3 vs the 2e-2 gate).  Biases enter as K=1
    bias-row matmuls so each layer needs a single whole-tile ReLU.
  - GRU with fp32 PSUM accumulators (r/z gates, i_n, h_n, and the
    output MLP's first layer persist across the 8 unrolled steps) and
    bf16 state tensors (hh, n, u, m, d1) so the moving matmul operands
    run at 1 cycle/row.  With m = -dlt = (z-1)*(hh-n), every
    accumulator updates as "psum += W@m" (weights kept POSITIVE on the
    host), and the x-recurrence folds through the output MLP:
    gi_x += (W_ihx @ W23.T) @ relu(pd1).  m is computed by a single
    fused scalar_tensor_tensor op.
  - all 8 waypoints accumulate into one SBUF tile [3, 8, 32]; a single
    output DMA at the very end (no per-step DMAs on the tail).
"""

import numpy as np
import ml_dtypes
from contextlib import ExitStack

N_CORES = 8
B = 256
B_SH = B // N_CORES       # 32 batch rows per core
C = 512                   # channels
S = 256                   # spatial 16*16
HID = 64
T = 8                     # pred_len
N_ZDMA = 8                # z DMAs per core (4 batches each)
B_BLK = B_SH // N_ZDMA    # batches per z DMA

# packed-constants layout: (name, partitions, cols); column offsets accumulate
_PACK = [
    ("biases", 128, 8),     # jb1 x4, jb2 x2, jb3, jb4
    ("whhbt", 65, 192),     # [W_hh.T; (0...0, b_hh_n)]           (init mms)
    ("wgobt", 4, 192),      # [W_ih[:,3:6].T; (b_rz_sum, b_ih_n)] (init mms)
    ("goalones", 4, B_SH),  # [goal.T; ones]
    ("ow1bt", 65, 4),       # [oW1.T; ob1]                        (init pd1)
    ("whhpt", 64, 192),     # +W_hh.T            (incremental, applied to m)
    ("wixobt", 33, 192),    # x-path folded through d1: rows0:4 =
                            #   W23 @ W_ihx.T, row32 = W_ihx @ b23
    ("ow1pt", 64, 4),       # +oW1.T             (incremental pd1, against m)
    ("ow23bt", 33, 3),      # rows0:4 = W23, row32 = b23  (output dx)
]
_OFF = {}
_ncol = 0
for _n, _p, _c in _PACK:
    _OFF[_n] = _ncol
    _ncol += _c
PACK_COLS = _ncol

_CACHE: dict = {}

# Engine-executing compute ops run strictly in order on their engine, so a
# wait on the instruction's OWN engine-execution semaphore (Tile emits one
# for every same-engine RAW/WAR edge) is satisfied by program order; in the
# timing model it costs the producer's pipeline tail + sem propagation
# (~160 ns per edge).  DMA and seq-only instructions check waits at the
# sequencer, which runs ahead of the engine — those must keep their waits.
_ENGINE_OPS = {
    "InstActivation", "InstTensorTensor", "InstTensorScalarPtr",
    "InstTensorReduce", "InstTensorCopy", "InstMemset", "InstMatmult",
    "InstLdweights", "InstLoadActFuncSet",
}


def _merge_eventsem_waits(nc):
    """Drop Activation ops' waits on the ACT exec-sem (WAW/WAR over the
    work-tile rotation; safe under in-order execution, unlike the RAW
    same-engine waits on DVE), then fold each SEQ-blocking EventSemaphore's
    wait into the next same-engine compute op's freed wait slot so the op
    parks in the engine wait queue instead of stalling the sequencer."""
    merged = 0
    for blk in nc.m.functions[0].blocks:
        insts = blk.instructions
        for x in insts:
            if type(x).__name__ != "InstActivation":
                continue
            si = x.sync_info
            if si is None or not si.on_wait:
                continue
            keep = [w for w in si.on_wait
                    if "ant_name='Activation_" not in str(w)]
            if len(keep) != len(si.on_wait):
                si.on_wait = keep
        pending = {}  # engine -> EventSemaphore inst with waits to place
        for x in insts:
            eng = str(getattr(x, "engine", "")).split(".")[-1]
            nm = type(x).__name__
            if nm == "InstEventSemaphore":
                si = x.sync_info
                if si is not None and si.on_wait and not si.on_update:
                    pending[eng] = x
                else:
                    pending.pop(eng, None)
                continue
            if eng in pending and nm in _ENGINE_OPS:
                es = pending.pop(eng)
                si = x.sync_info
                if si is not None and not si.on_wait:
                    si.on_wait = list(es.sync_info.on_wait)
                    es.sync_info.on_wait = []
                    merged += 1
            elif eng in pending:
                pending.pop(eng, None)
    return merged


def _strip_same_engine_waits(nc):
    stripped = 0
    for blk in nc.m.functions[0].blocks:
        for x in blk.instructions:
            if type(x).__name__ not in _ENGINE_OPS:
                continue
            si = x.sync_info
            if si is None or not si.on_wait:
                continue
            eng = str(getattr(x, "engine", "")).split(".")[-1]
            keep = []
            for w in si.on_wait:
                s = str(w)
                name = s.split("ant_name='")[1].split("'")[0] \
                    if "ant_name='" in s else ""
                if (name.startswith(eng + "_")
                        and "sem-ge-imm" in s
                        and not name.startswith("DMAHW")):
                    stripped += 1
                else:
                    keep.append(w)
            if len(keep) != len(si.on_wait):
                si.on_wait = keep
    return stripped


def _build_program():
    import concourse.bacc as bacc
    import concourse.tile as tile
    from concourse import mybir

    f32 = mybir.dt.float32
    bf16 = mybir.dt.bfloat16
    f8 = mybir.dt.float8e3
    AF = mybir.ActivationFunctionType
    AX = mybir.AxisListType
    ALU = mybir.AluOpType

    nc = bacc.Bacc("TRN2", target_bir_lowering=False, debug=False)

    zq = nc.dram_tensor("zq", [128, B_SH * 2 * 4 * 128], f8, kind="ExternalInput")
    one8_d = nc.dram_tensor("one8", [128, 1], f8, kind="ExternalInput")
    brows_d = nc.dram_tensor("brows", [1, 960], bf16, kind="ExternalInput")
    jw1t = nc.dram_tensor("jw1t", [512, 512], f8, kind="ExternalInput")
    jw2t = nc.dram_tensor("jw2t", [512, 256], f8, kind="ExternalInput")
    jw3t = nc.dram_tensor("jw3t", [256, 128], f8, kind="ExternalInput")
    jw4t = nc.dram_tensor("jw4t", [128, 64], f8, kind="ExternalInput")
    wpack = nc.dram_tensor("wpack", [128, PACK_COLS], bf16, kind="ExternalInput")
    out_d = nc.dram_tensor("out", [3, T * B_SH], f32, kind="ExternalOutput")

    with tile.TileContext(nc) as tc, ExitStack() as ctx:
        consts = ctx.enter_context(tc.tile_pool(name="consts", bufs=1))
        hpool = ctx.enter_context(tc.tile_pool(name="hpool", bufs=1))
        work = ctx.enter_context(tc.tile_pool(name="work", bufs=2))
        psum_mlp = ctx.enter_context(
            tc.tile_pool(name="psum_mlp", bufs=2, space="PSUM"))
        psum_gru = ctx.enter_context(
            tc.tile_pool(name="psum_gru", bufs=1, space="PSUM"))
        kw = dict(skip_group_check=True)

        # --- DMA queue (SP): z stream first (it gates everything), then
        # the weights in the order the MLP layers consume them.
        one8 = consts.tile([128, 1], f8)
        nc.vector.memset(one8, 1.0)
        zt = consts.tile([128, B_SH, 2, 4, 128], f8)
        zq_r = zq[:].rearrange("p (d r) -> d p r", d=N_ZDMA)
        zt_r = zt[:].rearrange("p (d b) h j c -> d p (b h j c)", d=N_ZDMA)
        nc.scalar.dma_start(out=zt_r[0], in_=zq_r[0])
        for d in range(1, N_ZDMA):
            nc.sync.dma_start(out=zt_r[d], in_=zq_r[d])
        brows = consts.tile([1, 960], bf16)
        nc.sync.dma_start(out=brows, in_=brows_d[:])

        w1 = consts.tile([128, 4, 512], f8)
        nc.sync.dma_start(
            out=w1, in_=jw1t[:].rearrange("(k p) m -> p k m", p=128))
        w2 = consts.tile([128, 4, 256], f8)
        nc.sync.dma_start(
            out=w2, in_=jw2t[:].rearrange("(k p) m -> p k m", p=128))
        w3 = consts.tile([128, 2, 128], f8)
        nc.sync.dma_start(
            out=w3, in_=jw3t[:].rearrange("(k p) m -> p k m", p=128))
        w4 = consts.tile([128, 64], f8)
        nc.sync.dma_start(out=w4, in_=jw4t[:])
        wp = consts.tile([128, PACK_COLS], bf16)
        nc.sync.dma_start(out=wp, in_=wpack[:])

        bs = wp[0:128, _OFF["biases"]:_OFF["biases"] + 8]
        whh = wp[0:65, _OFF["whhbt"]:_OFF["whhbt"] + 192]
        wgo = wp[0:4, _OFF["wgobt"]:_OFF["wgobt"] + 192]
        gl = wp[0:4, _OFF["goalones"]:_OFF["goalones"] + B_SH]
        ow1 = wp[0:65, _OFF["ow1bt"]:_OFF["ow1bt"] + 4]
        whhp = wp[0:64, _OFF["whhpt"]:_OFF["whhpt"] + 192]
        wixo = wp[0:33, _OFF["wixobt"]:_OFF["wixobt"] + 192]
        ow1p = wp[0:64, _OFF["ow1pt"]:_OFF["ow1pt"] + 4]
        ow23 = wp[0:33, _OFF["ow23bt"]:_OFF["ow23bt"] + 3]

        # ACT table warmup: sigmoid/tanh tables resident before the tail.
        warm = consts.tile([1, 1], f32)
        nc.vector.memset(warm, 0.0)
        nc.scalar.activation(warm, warm, AF.Sigmoid)
        nc.scalar.activation(warm, warm, AF.Tanh)
        # all-ones rhs column for the K=1 bias-row matmuls
        onef = consts.tile([1, B_SH], bf16)
        nc.vector.memset(onef, 1.0)

        # --- spatial reduce on the PE: hTp[c128, (j, b)] = sum_s z ---
        hTp = psum_mlp.tile([128, 4, B_SH], f32, tag="mlp")
        for b in range(B_SH):
            for j in range(4):
                for h in range(2):
                    nc.tensor.matmul(hTp[:, j, b:b + 1], zt[:, b, h, j, :],
                                     one8, start=(h == 0), stop=(h == 1), **kw)
        hT = hpool.tile([128, 4, B_SH], bf16)
        nc.vector.tensor_copy(hT, hTp)

        # --- join MLP (transposed, bf16): hN_T = relu(W @ h_T + b) ---
        # Biases enter as K=1 bias-row matmuls (brows x ones-column) so each
        # layer needs a single whole-tile Relu on ACT, and all matmuls of a
        # layer precede it (tile-granular tracking would otherwise serialize
        # chunk m+1's matmuls behind the relu read of chunk m).
        h1 = hpool.tile([128, 4, B_SH], bf16)
        pt1 = psum_mlp.tile([128, 4, B_SH], f32, tag="mlp")
        # bias matmuls first: they only need brows/onef, so they execute
        # while the weight DMAs are still in flight (PE runs in order).
        for m in range(4):
            nc.tensor.matmul(pt1[:, m, :], brows[0:1, m * 128:(m + 1) * 128],
                             onef, start=True, stop=False, **kw)
        for m in range(4):
            for k in range(4):
                nc.tensor.matmul(pt1[:, m, :], w1[:, k, m * 128:(m + 1) * 128],
                                 hT[:, k, :], start=False, stop=(k == 3), **kw)
        nc.vector.tensor_scalar_max(h1, pt1, 0.0)
        h2 = hpool.tile([128, 2, B_SH], bf16)
        pt2 = psum_mlp.tile([128, 2, B_SH], f32, tag="mlp")
        for m in range(2):
            nc.tensor.matmul(pt2[:, m, :],
                             brows[0:1, 512 + m * 128:512 + (m + 1) * 128],
                             onef, start=True, stop=False, **kw)
        for m in range(2):
            for k in range(4):
                nc.tensor.matmul(pt2[:, m, :], w2[:, k, m * 128:(m + 1) * 128],
                                 h1[:, k, :], start=False, stop=(k == 3), **kw)
        nc.vector.tensor_scalar_max(h2, pt2, 0.0)
        h3 = hpool.tile([128, B_SH], bf16)
        pt3 = psum_mlp.tile([128, B_SH], f32, tag="mlp")
        nc.tensor.matmul(pt3, brows[0:1, 768:896], onef,
                         start=True, stop=False, **kw)
        for k in range(2):
            nc.tensor.matmul(pt3, w3[:, k, :], h2[:, k, :],
                             start=False, stop=(k == 1), **kw)
        nc.vector.tensor_scalar_max(h3, pt3, 0.0)

        # hhg rows 0:64 = GRU hidden state (in-place across steps), row 64 = 1.
        hhg = hpool.tile([65, B_SH], bf16)
        nc.vector.memset(hhg[64:65, :], 1.0)
        pt4 = psum_mlp.tile([64, B_SH], f32, tag="mlp")
        nc.tensor.matmul(pt4, brows[0:1, 896:960], onef,
                         start=True, stop=False, **kw)
        nc.tensor.matmul(pt4, w4, h3, start=False, stop=True, **kw)
        nc.vector.tensor_scalar_max(hhg[0:64, :], pt4, 0.0)

        # d1g: relu(pd1) with ones row at partition 32 (engine-writable);
        # rows 4:32 stay zero so the K=33 matmuls see only d1 + bias.
        d1g = hpool.tile([33, B_SH], bf16)
        nc.vector.memset(d1g[0:33, :], 0.0)
        nc.vector.memset(d1g[32:33, :], 1.0)

        # --- GRU: persistent psum accumulators, 8 unrolled steps ---
        prz = psum_gru.tile([128, B_SH], f32, tag="prz")   # r/z gate pre-act
        pin = psum_gru.tile([64, B_SH], f32, tag="pin")    # i_n pre-act
        phn = psum_gru.tile([64, B_SH], f32, tag="phn")    # h_n pre-act
        pd1 = psum_gru.tile([4, B_SH], f32, tag="pd1")     # oW1@hh+ob1
        ptm = psum_gru.tile([64, B_SH], f32, tag="ptm")    # tanh input
        # goal-only init matmuls have no hhg dependency; the hhg ones follow.
        nc.tensor.matmul(prz, wgo[:, 0:128], gl, start=True, stop=False, **kw)
        nc.tensor.matmul(pin, wgo[:, 128:192], gl, start=True, stop=False, **kw)
        nc.tensor.matmul(prz, whh[:, 0:128], hhg, start=False, stop=False, **kw)
        nc.tensor.matmul(phn, whh[:, 128:192], hhg, start=True, stop=False, **kw)
        nc.tensor.matmul(pd1, ow1[0:65, :], hhg, start=True, stop=False, **kw)

        xall = hpool.tile([3, T, B_SH], f32)
        for t in range(T):
            last = t == T - 1
            # gate path: r first (it gates the tanh input); z off-chain
            # until the m_t fuse; d1 relu rides the ACT queue after tanh.
            r_t = work.tile([64, B_SH], f32, tag="r_t")
            nc.scalar.activation(r_t, prz[0:64, :], AF.Sigmoid)
            z_t = work.tile([64, B_SH], bf16, tag="z_t")
            nc.scalar.activation(z_t, prz[64:128, :], AF.Sigmoid)
            tmp = work.tile([64, B_SH], f32, tag="tmp")
            nc.vector.tensor_mul(tmp, r_t, phn)             # r * h_n
            nc.vector.tensor_add(ptm, tmp, pin)             # + i_n -> PSUM
            n_t = work.tile([64, B_SH], bf16, tag="n_t")
            nc.scalar.activation(n_t, ptm, AF.Tanh)
            u_t = work.tile([64, B_SH], bf16, tag="u_t")
            nc.vector.tensor_sub(u_t, hhg[0:64, :], n_t)    # hh - n
            m_t = work.tile([64, B_SH], bf16, tag="m_t")
            nc.vector.scalar_tensor_tensor(                 # (z-1)*(hh-n) = -d
                out=m_t, in0=z_t, scalar=1.0, in1=u_t,
                op0=ALU.subtract, op1=ALU.mult)

            # hh' = hh + m; pd1 first (it gates the output path), then
            # the other accumulators.
            nc.tensor.matmul(pd1, ow1p, m_t, start=False, stop=last, **kw)
            nc.vector.tensor_scalar_max(d1g[0:4, :], pd1, 0.0)  # d1(hh')
            if not last:
                nc.tensor.matmul(prz, whhp[:, 0:128], m_t,
                                 start=False, stop=False, **kw)
                nc.tensor.matmul(phn, whhp[:, 128:192], m_t,
                                 start=False, stop=(t == T - 2), **kw)
                nc.vector.tensor_add(hhg[0:64, :], hhg[0:64, :], m_t)
                # x-recurrence folded through d1g
                nc.tensor.matmul(prz, wixo[:, 0:128], d1g,
                                 start=False, stop=(t == T - 2), **kw)
                nc.tensor.matmul(pin, wixo[:, 128:192], d1g,
                                 start=False, stop=(t == T - 2), **kw)

            # x output (off the critical chain)
            pd3 = psum_gru.tile([3, B_SH], f32, tag="pd3")
            nc.tensor.matmul(pd3, ow23, d1g, start=True, stop=True)
            if t == 0:
                nc.vector.tensor_copy(xall[:, 0, :], pd3)
            else:
                nc.vector.tensor_add(xall[:, t, :], xall[:, t - 1, :], pd3)

        nc.sync.dma_start(
            out=out_d[:], in_=xall[:].rearrange("c t b -> c (t b)"))

    # NOTE: stripping ALL same-engine waits (_strip_same_engine_waits) saves
    # ~2.6 us in the cost model but produces WRONG RESULTS on hardware —
    # engine writes only become visible after the pipeline ack that the
    # semaphore guards, so same-engine RAW genuinely needs the sems.
    # _merge_eventsem_waits below is the SAFE subset: ACT-on-ACT waits are
    # WAW/WAR over the work-tile rotation (in-order engine => ordered), and
    # freeing that slot lets the cross-engine wait held by the preceding
    # SEQ-blocking EventSemaphore ride the instruction's own wait slot.
    _merge_eventsem_waits(nc)
    nc.compile()
    return nc


def _get_program():
    if "nc" not in _CACHE:
        _CACHE["nc"] = _build_program()
    return _CACHE["nc"]


def make_in_maps(**inputs) -> list[dict]:
    """Host-side packing + data-parallel sharding -> one in_map per core."""
    f = lambda a: np.ascontiguousarray(np.asarray(a, dtype=np.float32))
    z = f(inputs["z"]).reshape(B, C, S)
    gp = f(inputs["goal_point"])
    gps = f(inputs["goal_point_speed"])
    W_ih, W_hh = f(inputs["W_ih"]), f(inputs["W_hh"])
    b_ih, b_hh = f(inputs["b_ih"]), f(inputs["b_hh"])
    oW1, ob1 = f(inputs["oW1"]), f(inputs["ob1"])
    oW2, ob2 = f(inputs["oW2"]), f(inputs["ob2"])
    oW3, ob3 = f(inputs["oW3"]), f(inputs["ob3"])
    bf16 = ml_dtypes.bfloat16
    f8 = ml_dtypes.float8_e3m4

    # z -> fp8, spatial on partitions: zq[ps, b, h, j, cc] = z[b, 4cc+j, 128h+ps]
    # z[b, c, s] view as [b, cc, j, h, ps] (c = 4cc+j, s = 128h+ps)
    zv = z.reshape(B, 128, 4, 2, 128).transpose(4, 0, 3, 2, 1)  # [ps,b,h,j,cc]
    zq = np.ascontiguousarray(zv.astype(f8))  # [128, 256, 2, 4, 128]

    # layer-1 weight: fold the 1/S mean scale and the z-layout channel
    # permutation (partition p, chunk j <-> channel 4p+j).
    jw1t = f(inputs["jW1"]).T * np.float32(1.0 / S)
    perm = (4 * np.arange(128)[None, :] + np.arange(4)[:, None]).reshape(-1)
    jw1t = np.ascontiguousarray(jw1t[perm].astype(f8))
    jw2t = np.ascontiguousarray(f(inputs["jW2"]).T.astype(f8))
    jw3t = np.ascontiguousarray(f(inputs["jW3"]).T.astype(f8))
    jw4t = np.ascontiguousarray(f(inputs["jW4"]).T.astype(f8))

    # bias pack [128, 8]: jb1 (4 cols), jb2 (2), jb3 (1), jb4 (1, rows 0:64)
    biases = np.zeros((128, 8), np.float32)
    biases[:, 0:4] = f(inputs["jb1"]).reshape(4, 128).T
    biases[:, 4:6] = f(inputs["jb2"]).reshape(2, 128).T
    biases[:, 6] = f(inputs["jb3"])
    biases[0:64, 7] = f(inputs["jb4"])

    brow = np.concatenate([b_ih[0:128] + b_hh[0:128], b_ih[128:192]])
    wgobt = np.concatenate([W_ih[:, 3:6].T, brow[None, :]])  # [4, 192]
    brow2 = np.concatenate([np.zeros(128, np.float32), b_hh[128:192]])
    whhbt = np.concatenate([W_hh.T, brow2[None, :]])         # [65, 192]
    whhpt = W_hh.T                                           # [64, 192]

    ow1bt = np.concatenate([oW1.T, ob1[None, :]])            # [65, 4]
    ow1pt = oW1.T                                            # [64, 4]
    w23 = oW2.T @ oW3.T                                      # [4, 3]
    b23 = ob2 @ oW3.T + ob3                                  # [3]
    ow23bt = np.zeros((33, 3), np.float32)
    ow23bt[0:4] = w23
    ow23bt[32] = b23
    # x-recurrence folded through d1:  W_ihx @ dx = (W23 @ W_ihx.T).T@d1...
    wixobt = np.zeros((33, 192), np.float32)
    wixobt[0:4] = w23 @ W_ih[:, 0:3].T                       # [4, 192]
    wixobt[32] = W_ih[:, 0:3] @ b23                          # [192]

    goalT = np.stack([gp[:, 0, 3], gp[:, 1, 3], gps])        # [3, 256]
    one8 = np.ones((128, 1), f8)
    brows = np.concatenate(
        [f(inputs["jb1"]), f(inputs["jb2"]), f(inputs["jb3"]),
         f(inputs["jb4"])])[None, :].astype(bf16)            # [1, 960]


    segs = dict(biases=biases, whhbt=whhbt, wgobt=wgobt, ow1bt=ow1bt,
                whhpt=whhpt, wixobt=wixobt, ow1pt=ow1pt, ow23bt=ow23bt)
    in_maps = []
    for i in range(N_CORES):
        sl = slice(i * B_SH, (i + 1) * B_SH)
        go = np.concatenate(
            [goalT[:, sl], np.ones((1, B_SH), np.float32)])  # [4, 32]
        pack = np.zeros((128, PACK_COLS), np.float32)
        for name, parts, cols in _PACK:
            arr = go if name == "goalones" else segs[name]
            pack[0:parts, _OFF[name]:_OFF[name] + cols] = arr
        pack = pack.astype(bf16)
        in_maps.append(dict(
            zq=np.ascontiguousarray(zq[:, sl].reshape(128, -1)),
            one8=one8, brows=brows,
            jw1t=jw1t, jw2t=jw2t, jw3t=jw3t, jw4t=jw4t,
            wpack=pack,
        ))
    return in_maps


def unshard_out(results: list[dict]) -> np.ndarray:
    # per-core out [3, T*B_SH]: row c, col t*B_SH+b  ->  [B_SH, T, 3]
    parts = [r["out"].reshape(3, T, B_SH).transpose(2, 1, 0) for r in results]
    return np.ascontiguousarray(np.concatenate(parts, axis=0), dtype=np.float32)


def kernel(**inputs) -> np.ndarray:
    from concourse.bass_utils import run_bass_kernel_spmd

    nc = _get_program()
    in_maps = make_in_maps(**inputs)
    res = run_bass_kernel_spmd(nc, in_maps, core_ids=list(range(N_CORES)))
    return unshard_out(res.results)
